# revision 8
# baseline (speedup 1.0000x reference)
"""GNN message passing (graph conv) on 8 Trainium2 NeuronCores.

Math:  out = elu(segment_sum(vals * (x @ W.T + b)[cols], rows))
Trick: segment_sum(v * (W x[c] + b)) = W @ segment_sum(v * x[c]) + segsum(v) * b
so we aggregate raw x rows (gather + one-hot matmul) and apply the 128x128
linear AFTER aggregation.

Sharding: destination rows are split across 8 cores (12500 each). Edges are
bucketed (host-side) by (core, dest SUPERTILE of 512 rows, src window) and
padded to 128-edge blocks. x is replicated to every core in bf16.

Gather: the production `dma_gather` op (InstDMAGatherAnt) -- one instruction
per (supertile, window) gathers ~1-1.3k rows (0.34ns/descriptor Q7 gen; the
drain pipeline saturates at ~2.8ns/row with 4 SWDGE queues, which is the
kernel's wall). int16 indices force NWIN=4 source windows of 25000 rows.
Gathered row i lands at [i%128, i//128, :]: each 128-edge block is one
[128, 128] lhsT slab.

Supertiles (ST=4 dest tiles = 512 rows = one PSUM bank) amortize DVE/PE
per-op overheads: ONE dual-op tensor_scalar builds a [128, 512] one-hot for
a whole block (~320ns, 16-bit 4x DVE mode; iota is int16 because bf16
cannot represent 257..511 exactly), and ONE PE matmul (N=512) applies it.
ELU runs on [128, 512] tiles.

Per core / per dest supertile st (512 rows):
  G[p, blk, :]  = x[idx[blk*128+p], :]                (dma_gather, 4 queues)
  O_b[e, r512]  = vals[e,b] * (iota16[r] == rows[e,b])  (DVE, one per block)
  aggT[f, :]   += G_b^T @ O_b                         (PE, N=512, PSUM bank)
  z[r, o]       = aggT[:, tile]^T @ W^T + s[r]*b[o]   (4 tiles per st)
  out           = elu(z) = max(z, exp(min(z,0)) - 1)  (wide [128, 512] ops)
"""

import numpy as np
import ml_dtypes

BF16 = ml_dtypes.bfloat16

N_NODES = 100000
D = 128
NCORES = 8
RPC = N_NODES // NCORES          # rows per core = 12500
P = 128
ST = 4                           # dest tiles per supertile (512 rows)
STW = ST * P                     # 512
NST = (RPC + STW - 1) // STW     # 25 supertiles per core
TILES = (RPC + P - 1) // P       # 98 real tiles
RPAD = NST * STW                 # 12800 padded rows per core
NWIN = 4
WND = N_NODES // NWIN            # 25000 rows per source window (int16 safe)


def _build_program(nblk, nb_max):
    """nblk: [NST][NWIN] block counts (shared across cores)."""
    import concourse.tile as tile
    from concourse import bacc, mybir
    from contextlib import ExitStack

    f32 = mybir.dt.float32
    bf16 = mybir.dt.bfloat16
    i16 = mybir.dt.int16
    i32 = mybir.dt.int32

    # per-supertile block layout: windows in order
    blkoff = [[0] * NWIN for _ in range(NST)]
    stblk = [0] * NST
    for st in range(NST):
        off = 0
        for w in range(NWIN):
            blkoff[st][w] = off
            off += nblk[st][w]
        stblk[st] = off
    NB = nb_max

    nc = bacc.Bacc("TRN2", target_bir_lowering=False, num_swdge_queues=4)

    x_d = nc.declare_dram_parameter("x", [N_NODES, D], bf16, isOutput=False)
    wt_d = nc.declare_dram_parameter("wT", [D, D], bf16, isOutput=False)
    b_d = nc.declare_dram_parameter("bvec", [1, D], bf16, isOutput=False)
    idx_d = nc.declare_dram_parameter("idx", [NST, P, NB * 8], i16, isOutput=False)
    rv_d = nc.declare_dram_parameter("rv", [NST, P, 2 * NB], f32, isOutput=False)
    s_d = nc.declare_dram_parameter("svec", [NST, 1, STW], bf16, isOutput=False)
    out_d = nc.declare_dram_parameter("out", [RPAD, D], f32, isOutput=True)

    with ExitStack() as ctx:
        tc = ctx.enter_context(tile.TileContext(nc))
        const = ctx.enter_context(tc.tile_pool(name="const", bufs=1))
        edges = ctx.enter_context(tc.tile_pool(name="edges", bufs=3))
        gbuf = ctx.enter_context(tc.tile_pool(name="gbuf", bufs=3))
        ohot = ctx.enter_context(tc.tile_pool(name="ohot", bufs=6))
        work = ctx.enter_context(tc.tile_pool(name="work", bufs=3))
        resp = ctx.enter_context(tc.tile_pool(name="resp", bufs=2))
        psum_a = ctx.enter_context(tc.tile_pool(name="psum_a", bufs=3, space="PSUM"))
        psum_b = ctx.enter_context(tc.tile_pool(name="psum_b", bufs=2, space="PSUM"))

        wt_sb = const.tile([D, D], bf16)
        nc.sync.dma_start(wt_sb[:], wt_d[:])
        b_sb = const.tile([1, D], bf16)
        nc.sync.dma_start(b_sb[:], b_d[:])
        iota_i = const.tile([P, STW], i32)
        nc.gpsimd.iota(iota_i[:], pattern=[[1, STW]], base=0, channel_multiplier=0)
        iota_s = const.tile([P, STW], i16)
        nc.vector.tensor_copy(iota_s[:], iota_i[:])

        for st in range(NST):
            idx_t = edges.tile([P, NB * 8], i16, tag="idx")
            nc.sync.dma_start(idx_t[:], idx_d[st])
            rv_t = edges.tile([P, 2 * NB], f32, tag="rv")
            nc.sync.dma_start(rv_t[:], rv_d[st])
            s_t = edges.tile([1, STW], bf16, tag="s")
            nc.sync.dma_start(s_t[:], s_d[st])

            g_t = gbuf.tile([P, NB, P], bf16)
            for w in range(NWIN):
                nb_w = nblk[st][w]
                off = blkoff[st][w]
                nc.gpsimd.dma_gather(
                    g_t[:, off : off + nb_w, :],
                    x_d[w * WND : (w + 1) * WND, :],
                    idx_t[:, off * 8 : (off + nb_w) * 8],
                    nb_w * P,
                    nb_w * P,
                    P,
                    queue_num=w,
                    single_packet=False,
                )

            agg_t = psum_a.tile([P, STW], f32, space="PSUM")
            nbs = stblk[st]
            for blk in range(nbs):
                o_c = ohot.tile([P, STW], bf16)
                nc.vector.tensor_scalar(
                    out=o_c[:],
                    in0=iota_s[:],
                    scalar1=rv_t[:, blk : blk + 1],
                    scalar2=rv_t[:, NB + blk : NB + blk + 1],
                    op0=mybir.AluOpType.is_equal,
                    op1=mybir.AluOpType.mult,
                )
                nc.tensor.matmul(
                    agg_t[:],
                    lhsT=g_t[:, blk, :],
                    rhs=o_c[:],
                    start=(blk == 0),
                    stop=(blk == nbs - 1),
                )

            agg_sb = work.tile([P, STW], bf16, tag="aggT")
            nc.vector.tensor_copy(agg_sb[:], agg_t[:])
            z_t = psum_b.tile([P, STW], f32, space="PSUM")
            for t4 in range(ST):
                sl = slice(t4 * P, (t4 + 1) * P)
                nc.tensor.matmul(
                    z_t[:, sl], lhsT=agg_sb[:, sl], rhs=wt_sb[:],
                    start=True, stop=False,
                )
                nc.tensor.matmul(
                    z_t[:, sl], lhsT=s_t[:, sl], rhs=b_sb[:],
                    start=False, stop=True,
                )

            # elu(z) = max(z, exp(min(z,0)) - 1), on [128, 512] tiles
            zmin = work.tile([P, STW], bf16, tag="zmin")
            nc.vector.tensor_scalar_min(zmin[:], z_t[:], 0.0)
            ez = work.tile([P, STW], bf16, tag="ez")
            nc.scalar.activation(ez[:], zmin[:], mybir.ActivationFunctionType.Exp)
            em1 = work.tile([P, STW], bf16, tag="em1")
            nc.vector.tensor_scalar_add(em1[:], ez[:], -1.0)
            res_t = resp.tile([P, STW], f32)
            nc.vector.tensor_tensor(
                out=res_t[:], in0=z_t[:], in1=em1[:], op=mybir.AluOpType.max
            )
            for t4 in range(ST):
                tg = st * ST + t4
                if tg < TILES:
                    nc.sync.dma_start(
                        out_d[tg * P : (tg + 1) * P, :],
                        res_t[:, t4 * P : (t4 + 1) * P],
                    )

    nc.compile()
    return nc


def _prep_inputs(x, W, b, adj_rows, adj_cols, adj_vals):
    """Host-side edge bucketing: group edges by (core, dest supertile, src
    window), pad each bucket to 128-edge blocks with block counts shared
    across cores, and lay out per-supertile gather indices / one-hot
    scalars."""
    rows = np.ascontiguousarray(adj_rows).astype(np.int64)
    cols = np.ascontiguousarray(adj_cols).astype(np.int64)
    vals = np.ascontiguousarray(adj_vals).astype(np.float32)

    core = rows // RPC
    local = rows - core * RPC
    stl = local // STW                     # supertile 0..24
    rloc = (local - stl * STW).astype(np.float32)   # row within supertile
    w = cols // WND

    key = (core * NST + stl) * NWIN + w
    order = np.argsort(key, kind="stable")
    k_s = key[order]
    c_s = cols[order]
    v_s = vals[order]
    r_s = rloc[order]

    nbuckets = NCORES * NST * NWIN
    cnt = np.bincount(k_s, minlength=nbuckets).reshape(NCORES, NST, NWIN)
    nblk_arr = (cnt.max(axis=0) + P - 1) // P          # [NST, NWIN] shared
    nblk = [[int(nblk_arr[s, wi]) for wi in range(NWIN)] for s in range(NST)]

    blkoff = np.zeros((NST, NWIN), np.int64)
    stblk = np.zeros(NST, np.int64)
    for s in range(NST):
        off = 0
        for wi in range(NWIN):
            blkoff[s, wi] = off
            off += nblk_arr[s, wi]
        stblk[s] = off
    nb_max = int(stblk.max())
    NB = nb_max

    starts = np.zeros(nbuckets, np.int64)
    starts[1:] = np.cumsum(cnt.reshape(-1))[:-1]
    pos = np.arange(len(k_s), dtype=np.int64) - starts[k_s]

    core_s = k_s // (NST * NWIN)
    sw_s = k_s % (NST * NWIN)
    st_g = sw_s // NWIN
    w_g = sw_s % NWIN
    slot = (blkoff[st_g, w_g] * P) + pos               # within-supertile slot
    dest = (core_s * NST + st_g) * (NB * P) + slot

    tot = NCORES * NST * NB * P
    idx_pad = np.zeros(tot, np.int16)
    val_pad = np.zeros(tot, np.float32)
    row_pad = np.zeros(tot, np.float32)
    idx_pad[dest] = (c_s - w_g * WND).astype(np.int16)
    val_pad[dest] = v_s
    row_pad[dest] = r_s

    # idx wrap: slot i -> [16k + i%16, i//16], k = 0..7
    idx_g = idx_pad.reshape(NCORES, NST, NB, 8, 16)
    idx_g = idx_g.transpose(0, 1, 4, 2, 3).reshape(NCORES, NST, 16, NB * 8)
    idx_full = np.tile(idx_g, (1, 1, 8, 1))

    def bshape(a):
        return np.ascontiguousarray(
            a.reshape(NCORES, NST, NB, P).transpose(0, 1, 3, 2)
        )

    rows_a = bshape(row_pad)
    vals_a = bshape(val_pad)
    rv = np.concatenate([rows_a, vals_a], axis=3)      # [NCORES, NST, P, 2NB]

    s_full = np.bincount(
        np.ascontiguousarray(adj_rows), weights=adj_vals.astype(np.float64),
        minlength=N_NODES,
    ).astype(np.float32)
    s_pad = np.zeros(NCORES * RPAD, dtype=np.float32)
    s_pad.reshape(NCORES, RPAD)[:, :RPC] = s_full.reshape(NCORES, RPC)
    s_a = s_pad.reshape(NCORES, NST, 1, STW).astype(BF16)

    x_g = np.ascontiguousarray(x).astype(BF16)
    wt_g = np.ascontiguousarray(W.T).astype(BF16)
    b_g = np.ascontiguousarray(b).reshape(1, D).astype(BF16)

    in_maps = []
    for i in range(NCORES):
        in_maps.append(
            {
                "x": x_g,
                "wT": wt_g,
                "bvec": b_g,
                "idx": np.ascontiguousarray(idx_full[i]),
                "rv": np.ascontiguousarray(rv[i]),
                "svec": s_a[i],
            }
        )
    return in_maps, nblk, nb_max


_CACHE = {}


def _run(in_maps, nblk, nb_max, trace=False):
    from concourse.bass_utils import run_bass_kernel_spmd

    key = (tuple(map(tuple, nblk)), nb_max)
    if key not in _CACHE:
        _CACHE[key] = _build_program(nblk, nb_max)
    nc = _CACHE[key]
    return run_bass_kernel_spmd(nc, in_maps, list(range(NCORES)), trace=trace)


def kernel(x, W, b, adj_rows, adj_cols, adj_vals, trace=False, _return_raw=False):
    x = np.asarray(x)
    in_maps, nblk, nb_max = _prep_inputs(
        x, np.asarray(W), np.asarray(b), np.asarray(adj_rows),
        np.asarray(adj_cols), np.asarray(adj_vals),
    )
    res = _run(in_maps, nblk, nb_max, trace=trace)
    outs = [res.results[i]["out"][:RPC] for i in range(NCORES)]
    full = np.concatenate(outs, axis=0).astype(np.float32)
    if _return_raw:
        return full, res
    return full
